# revision 20
# baseline (speedup 1.0000x reference)
# MoE (16 experts, top-4, shared SwiGLU MLP) on 8 Trainium2 NeuronCores.
#
# Strategy (expert-parallel, dense expert compute):
#   * Each core owns E/8 = 2 experts and HS/8 = 512 of the shared hidden dim.
#   * Router (fp32, full precision) is replicated on every core; per-core the
#     router weight columns are permuted so that "columns 0,1" of the combine
#     weights are this core's experts (softmax/top-k are permutation
#     invariant), letting all 8 cores run the identical SPMD program.
#   * Expert MLPs run densely over all T tokens in bf16 (fp32 PSUM accum);
#     the dense combine weight cw (zero for unselected experts) is applied as
#     a per-token scale when accumulating the expert output.
#   * Each core emits a partial [T, D] fp32 output (its 2 experts + its
#     shared-hidden slice); the host sums the 8 partials.
#   * k/(k+1) and 1/(k+1) blend factors are folded into we2 / ws2 on host.
import numpy as np
import ml_dtypes
from contextlib import ExitStack

import concourse.bass as bass
import concourse.tile as tile
from concourse import mybir
from concourse.bass_utils import run_bass_kernel_spmd

# ---------------------------------------------------------------- constants
T, D, E, H, HS = 4096, 2048, 16, 1024, 4096
TOP_K = 4
NCORES = 8
EPC = E // NCORES        # experts per core = 2
HSL = HS // NCORES       # shared hidden slice = 512
TB = 1024                # token block
NTB = T // TB            # 4
NDC = D // 128           # 16 contraction chunks
NHT_E = H // 128         # 8 expert hidden tiles
NHT_S = HSL // 128       # 4 shared hidden tiles
NDD = D // 512           # 4 output column tiles
NDP = D // 1024          # 2 output column pairs (N=1024 matmuls)
NTT = T // 128           # 32 token tiles
BF = ml_dtypes.bfloat16

f32 = mybir.dt.float32
bf16 = mybir.dt.bfloat16
AF = mybir.ActivationFunctionType
ALU = mybir.AluOpType

MAX_WAITS = 1  # this walrus build supports one sync-wait command per instruction


def _split_sync_waits(nc, max_waits: int = MAX_WAITS):
    """Hoist excess sync waits onto NOPs inserted before the instruction."""
    for f in nc.m.functions:
        for blk in f.blocks:
            insts = list(blk.instructions)
            for idx in range(len(insts) - 1, -1, -1):
                inst = insts[idx]
                si = inst.sync_info
                if si is None or len(si.on_wait) <= max_waits:
                    continue
                waits = list(si.on_wait)
                keep = waits[-max_waits:]
                extra = waits[:-max_waits]
                inst.sync_info = mybir.SyncInfo(
                    on_wait=keep, on_update=list(si.on_update)
                )
                nops = []
                for i in range(0, len(extra), max_waits):
                    nop = mybir.InstNoOp(
                        name=nc.get_next_instruction_name(), ins=[], outs=[]
                    )
                    nop.engine = inst.engine
                    nop.sync_info = mybir.SyncInfo(
                        on_wait=extra[i : i + max_waits], on_update=[]
                    )
                    nc.register_instruction(nop, overwrite=True)
                    nops.append(nop)
                for j, nop in enumerate(nops):
                    blk.instructions.insert(idx + j, nop)


class _TileContext(tile.TileContext):
    def __exit__(self, exc_type, exc_value, traceback):
        ret = super().__exit__(exc_type, exc_value, traceback)
        if exc_type is None:
            _split_sync_waits(self.nc)
        return ret


# ---------------------------------------------------------------- device IR
def _emit_router(nc, tc, P, rxt_bufs=6):
    """Router: fp32 logits -> dense renormalized top-4 combine weights."""
    with (
        tc.tile_pool(name="rxt", bufs=rxt_bufs) as rxt,
        tc.tile_pool(name="rtw", bufs=2) as rtw,
    ):
        for tt in range(NTT):
            xt_t = rxt.tile([128, NDC, 128], f32, tag="xt")
            nc.sync.dma_start(xt_t[:], P["xt32"][tt])
            ps = P["psum"].tile([128, E], f32, tag="ps")
            for c in range(NDC):
                nc.tensor.matmul(
                    ps[:], lhsT=xt_t[:, c, :], rhs=P["wr_sb"][:, c, :],
                    start=(c == 0), stop=(c == NDC - 1),
                )
            l = rtw.tile([128, E], f32, tag="l")
            nc.vector.tensor_copy(l[:], ps[:])
            m8 = rtw.tile([128, 8], f32, tag="m8")
            nc.vector.max(m8[:], l[:])
            negm1 = rtw.tile([128, 1], f32, tag="negm1")
            nc.vector.tensor_scalar(negm1[:], m8[:, 0:1], -1.0, None, ALU.mult)
            e8 = rtw.tile([128, 8], f32, tag="e8")
            nc.scalar.activation(e8[:], m8[:], AF.Exp, bias=negm1[:], scale=1.0)
            sx = rtw.tile([128, E], f32, tag="sx")
            nc.scalar.activation(sx[:], l[:], AF.Exp, bias=negm1[:], scale=1.0)
            den = rtw.tile([128, 1], f32, tag="den")
            nc.vector.tensor_reduce(
                den[:], e8[:, 0:TOP_K], axis=mybir.AxisListType.X, op=ALU.add
            )
            rec = rtw.tile([128, 1], f32, tag="rec")
            nc.vector.reciprocal(rec[:], den[:])
            mask = rtw.tile([128, E], f32, tag="mask")
            nc.vector.tensor_scalar(
                mask[:], l[:], m8[:, TOP_K - 1 : TOP_K], None, ALU.is_ge
            )
            nc.vector.scalar_tensor_tensor(
                P["cw_sb"][:, tt, :], sx[:], rec[:], mask[:], ALU.mult, ALU.mult
            )
            nc.scalar.dma_start(P["cwdbg"][tt], P["cw_sb"][:, tt, :])


def _emit_main(nc, tc, P):
    """Blockwise SwiGLU: shared slice + 2 experts, dense over tokens.
    N=512 moving operands (PSUM bank limit); s2 (shared second-layer
    weights) stays SBUF-resident."""
    psum, cw_sb = P["psum"], P["cw_sb"]
    xt16, w1, w3, w2 = P["xt16"], P["w1"], P["w3"], P["w2"]
    s1, s3, outp = P["s1"], P["s3"], P["outp"]
    s2_sb = P["s2_sb"]
    xtp, wlp, wrp2 = P["xtp"], P["wlp"], P["wrp2"]
    gp, sap, accp = P["gp"], P["sap"], P["accp"]

    for tb in range(NTB):
        xtb = xtp.tile([128, NDC, TB], bf16, tag="xtb")
        nc.sync.dma_start(xtb[:], xt16[:, :, tb * TB : (tb + 1) * TB])

        specs = [("s", 0, NHT_S), ("e", 0, NHT_E), ("e", 1, NHT_E)]
        g_tiles = []
        for kind, e, nht in specs:
            for ht in range(nht):
                wa = wlp.tile([128, NDC, 128], bf16, tag="wl")
                wb = wlp.tile([128, NDC, 128], bf16, tag="wl")
                if kind == "s":
                    nc.sync.dma_start(wa[:], s1[ht])
                    nc.sync.dma_start(wb[:], s3[ht])
                else:
                    nc.sync.dma_start(wa[:], w1[e, ht])
                    nc.sync.dma_start(wb[:], w3[e, ht])
                g = gp.tile([128, TB], bf16, tag="g")
                for th in range(TB // 512):
                    sl = slice(th * 512, (th + 1) * 512)
                    psA = psum.tile([128, 512], f32, tag="ps")
                    psB = psum.tile([128, 512], f32, tag="ps")
                    for c in range(NDC):
                        nc.tensor.matmul(
                            psA[:], lhsT=wa[:, c, :], rhs=xtb[:, c, sl],
                            start=(c == 0), stop=(c == NDC - 1),
                        )
                        nc.tensor.matmul(
                            psB[:], lhsT=wb[:, c, :], rhs=xtb[:, c, sl],
                            start=(c == 0), stop=(c == NDC - 1),
                        )
                    sA = sap.tile([128, 512], bf16, tag="sA")
                    nc.scalar.activation(sA[:], psA[:], AF.Silu)
                    nc.vector.tensor_tensor(g[:, sl], sA[:], psB[:], ALU.mult)
                g_tiles.append(g)
        gs_s = g_tiles[:NHT_S]
        gs_e0 = g_tiles[NHT_S : NHT_S + NHT_E]
        gs_e1 = g_tiles[NHT_S + NHT_E :]

        for dd in range(NDD):
            r2e0 = wrp2.tile([128, NHT_E, 512], bf16, tag="wrhs")
            nc.sync.dma_start(r2e0[:], w2[0, dd])
            r2e1 = wrp2.tile([128, NHT_E, 512], bf16, tag="wrhs")
            nc.sync.dma_start(r2e1[:], w2[1, dd])
            for i in range(TB // 128):
                tt = tb * (TB // 128) + i
                psS = psum.tile([128, 512], f32, tag="ps")
                for c in range(NHT_S):
                    nc.tensor.matmul(
                        psS[:], lhsT=gs_s[c][:, i * 128 : (i + 1) * 128],
                        rhs=s2_sb[:, dd, c, :],
                        start=(c == 0), stop=(c == NHT_S - 1),
                    )
                psE0 = psum.tile([128, 512], f32, tag="ps")
                for c in range(NHT_E):
                    nc.tensor.matmul(
                        psE0[:], lhsT=gs_e0[c][:, i * 128 : (i + 1) * 128],
                        rhs=r2e0[:, c, :],
                        start=(c == 0), stop=(c == NHT_E - 1),
                    )
                psE1 = psum.tile([128, 512], f32, tag="ps")
                for c in range(NHT_E):
                    nc.tensor.matmul(
                        psE1[:], lhsT=gs_e1[c][:, i * 128 : (i + 1) * 128],
                        rhs=r2e1[:, c, :],
                        start=(c == 0), stop=(c == NHT_E - 1),
                    )
                acc = accp.tile([128, 512], f32, tag="acc")
                nc.scalar.copy(acc[:], psS[:])
                nc.vector.scalar_tensor_tensor(
                    acc[:], psE0[:], cw_sb[:, tt, 0:1], acc[:], ALU.mult, ALU.add
                )
                nc.vector.scalar_tensor_tensor(
                    acc[:], psE1[:], cw_sb[:, tt, 1:2], acc[:], ALU.mult, ALU.add
                )
                r0 = tb * TB + i * 128
                nc.scalar.dma_start(
                    outp[r0 : r0 + 128, dd * 512 : (dd + 1) * 512], acc[:]
                )


def _build_bass(reps: int = 1):
    nc = bass.Bass(name="moe_ep")

    P = {}
    P["xt32"] = nc.dram_tensor("xt32", [NTT, 128, NDC, 128], f32, kind="ExternalInput")
    P["xt16"] = nc.dram_tensor("xt16", [128, NDC, T], bf16, kind="ExternalInput")
    wr = nc.dram_tensor("wr", [128, NDC, E], f32, kind="ExternalInput")
    P["w1"] = nc.dram_tensor("w1", [EPC, NHT_E, 128, NDC, 128], bf16, kind="ExternalInput")
    P["w3"] = nc.dram_tensor("w3", [EPC, NHT_E, 128, NDC, 128], bf16, kind="ExternalInput")
    P["w2"] = nc.dram_tensor("w2", [EPC, NDD, 128, NHT_E, 512], bf16, kind="ExternalInput")
    P["s1"] = nc.dram_tensor("s1", [NHT_S, 128, NDC, 128], bf16, kind="ExternalInput")
    P["s3"] = nc.dram_tensor("s3", [NHT_S, 128, NDC, 128], bf16, kind="ExternalInput")
    P["s2"] = nc.dram_tensor("s2", [128, NDD, NHT_S, 512], bf16, kind="ExternalInput")
    P["outp"] = nc.dram_tensor("outp", [T, D], f32, kind="ExternalOutput")
    P["cwdbg"] = nc.dram_tensor("cwdbg", [NTT, 128, E], f32, kind="ExternalOutput")

    with _TileContext(nc) as tc, ExitStack() as ctx:
        P["psum"] = ctx.enter_context(tc.tile_pool(name="psum", bufs=8, space="PSUM"))
        resp = ctx.enter_context(tc.tile_pool(name="res", bufs=1))
        P["wr_sb"] = resp.tile([128, NDC, E], f32, tag="wr", name="wr_sb")
        nc.sync.dma_start(P["wr_sb"][:], wr[:])
        P["cw_sb"] = resp.tile([128, NTT, E], f32, tag="cw", name="cw_sb")
        P["s2_sb"] = resp.tile([128, NDD, NHT_S, 512], bf16, tag="s2", name="s2_sb")
        nc.sync.dma_start(P["s2_sb"][:], P["s2"][:])

        for rep in range(reps):
            _emit_router(nc, tc, P, rxt_bufs=6 if rep == 0 else 2)
            if rep == 0:
                P["xtp"] = ctx.enter_context(tc.tile_pool(name="xtb", bufs=1))
                P["wlp"] = ctx.enter_context(tc.tile_pool(name="wl", bufs=8))
                P["wrp2"] = ctx.enter_context(tc.tile_pool(name="wrhs", bufs=6))
                P["gp"] = ctx.enter_context(tc.tile_pool(name="g", bufs=21))
                P["sap"] = ctx.enter_context(tc.tile_pool(name="sA", bufs=3))
                P["accp"] = ctx.enter_context(tc.tile_pool(name="acc", bufs=3))
            _emit_main(nc, tc, P)
    return nc


_BUILT = None


def _get_bass():
    global _BUILT
    if _BUILT is None:
        _BUILT = _build_bass()
    return _BUILT


# ---------------------------------------------------------------- host side
def _prep_in_maps(x, w_router, we1, we3, we2, ws1, ws3, ws2):
    x = np.asarray(x, np.float32)
    w_router = np.asarray(w_router, np.float32)
    xt = np.ascontiguousarray(x.reshape(T, D))
    xT = np.ascontiguousarray(xt.T)  # [D, T]
    xt16 = xT.reshape(NDC, 128, T).transpose(1, 0, 2).astype(BF)
    xt32 = np.ascontiguousarray(
        xT.reshape(NDC, 128, NTT, 128).transpose(2, 1, 0, 3)
    )
    blend_e = TOP_K / (TOP_K + 1.0)
    blend_s = 1.0 / (TOP_K + 1.0)
    in_maps = []
    for c in range(NCORES):
        e0 = EPC * c
        mine = list(range(e0, e0 + EPC))
        perm = mine + [e for e in range(E) if e not in mine]
        wr_t = np.ascontiguousarray(
            w_router[:, perm].reshape(NDC, 128, E).transpose(1, 0, 2)
        )
        w1_t = (
            np.asarray(we1[e0 : e0 + EPC], np.float32)
            .reshape(EPC, NDC, 128, NHT_E, 128)
            .transpose(0, 3, 2, 1, 4)
            .astype(BF)
        )
        w3_t = (
            np.asarray(we3[e0 : e0 + EPC], np.float32)
            .reshape(EPC, NDC, 128, NHT_E, 128)
            .transpose(0, 3, 2, 1, 4)
            .astype(BF)
        )
        w2_t = (
            (np.asarray(we2[e0 : e0 + EPC], np.float32) * blend_e)
            .reshape(EPC, NHT_E, 128, NDD, 512)
            .transpose(0, 3, 2, 1, 4)
            .astype(BF)
        )
        s1_t = (
            np.asarray(ws1[:, c * HSL : (c + 1) * HSL], np.float32)
            .reshape(NDC, 128, NHT_S, 128)
            .transpose(2, 1, 0, 3)
            .astype(BF)
        )
        s3_t = (
            np.asarray(ws3[:, c * HSL : (c + 1) * HSL], np.float32)
            .reshape(NDC, 128, NHT_S, 128)
            .transpose(2, 1, 0, 3)
            .astype(BF)
        )
        s2_t = (
            (np.asarray(ws2[c * HSL : (c + 1) * HSL], np.float32) * blend_s)
            .reshape(NHT_S, 128, NDD, 512)
            .transpose(1, 2, 0, 3)
            .astype(BF)
        )
        in_maps.append(
            {
                "xt32": xt32, "xt16": xt16, "wr": wr_t,
                "w1": np.ascontiguousarray(w1_t),
                "w3": np.ascontiguousarray(w3_t),
                "w2": np.ascontiguousarray(w2_t),
                "s1": np.ascontiguousarray(s1_t),
                "s3": np.ascontiguousarray(s3_t),
                "s2": np.ascontiguousarray(s2_t),
            }
        )
    return in_maps


def kernel(x, w_router, we1, we3, we2, ws1, ws3, ws2):
    nc = _get_bass()
    in_maps = _prep_in_maps(x, w_router, we1, we3, we2, ws1, ws3, ws2)
    res = run_bass_kernel_spmd(nc, in_maps, list(range(NCORES)))
    out = res.results[0]["outp"].astype(np.float32)
    for c in range(1, NCORES):
        out = out + res.results[c]["outp"]
    return np.ascontiguousarray(out.reshape(np.asarray(x).shape)).astype(np.float32)
